# revision 34
# baseline (speedup 1.0000x reference)
"""Trainium2 Bass kernel for nn_Aligner: dual transformer encoder + pairwise
log-softmax alignment. Data-parallel over batch B=8 across 8 NeuronCores
(one batch element per core); encoder weights + embedding tables replicated
per core; embedding lookup on-device via indirect DMA.

Numerics: fp8e4 (e4m3) DoubleRow matmuls for all dense projections
(QKV, V, O, FF1, FF2, final cross) contracting 256 per pass; fp16 for
attention scores/probs/attn@v; fp32 PSUM accumulation and fp32 stats.

Scale convention: weights and the encoder activation stream are stored at
16x their true value (keeps fp8e4 operands out of the denormal range);
PSUM results of fp8 GEMMs are 256x and get a 1/16 or 1/256 descale folded
into the existing DVE/ACT consumer op. LayerNorm is scale-invariant, so the
16x residual stream needs no extra descale passes.

Scheduling: the two encoders are emitted as interleaved generators (phase
granularity) so that text-encoder matmuls fill the unit encoder's pipeline
stalls (LayerNorm chains, softmax waits) and vice versa — the PE executes
its queue in order, so bubble-filling work must be interleaved in program
order.
"""

import numpy as np
import ml_dtypes
from contextlib import ExitStack

import concourse.bass as bass
import concourse.tile as tile
from concourse import bacc, mybir
from concourse.bass_utils import run_bass_kernel_spmd
from concourse.masks import make_identity

# model constants (hardcoded per problem spec)
D, H, DH, FFD, L = 1024, 8, 128, 2048, 3
KD = D // 128          # 8 feature tiles
KF = FFD // 128        # 16 ff tiles
T, U, B = 256, 1024, 8
LN_EPS, TEMP, NEG = 1e-5, 5e-4, -1e9
SCALE = float(1.0 / np.sqrt(DH).astype(np.float32))
SC = 16.0              # fp8 storage scale for weights + stream
SC2 = SC * SC

F16, F32, F8, I32 = mybir.dt.float16, mybir.dt.float32, mybir.dt.float8e4, mybir.dt.int32
AOP = mybir.AluOpType
AF = mybir.ActivationFunctionType
DR = mybir.MatmulPerfMode.DoubleRow
NP_F8 = ml_dtypes.float8_e4m3

TRACE = False
TRACE_DIR = None
LAST_RESULTS = None
_CACHE = {}


def _nblocks(S):
    return [(n0, min(512, S - n0)) for n0 in range(0, S, 512)]


def _npairs(S):
    """n-blocks grouped in pairs so consecutive matmuls share a stationary
    operand (hides LDWEIGHTS under the matmul stream)."""
    nb = _nblocks(S)
    return [nb[i:i + 2] for i in range(0, len(nb), 2)]


class _Ctx:
    pass


def _emit_cross(g, ufT, uts):
    """Cross matmul (256x) + exp row-sums for the given u-tiles; the
    log-softmax normalization is finished by _emit_final (one batched Ln).
    Requires g.tfT / g.growhl (text encoder finished)."""
    nc, p = g.nc, g.pools
    assert g.growhl is not None
    for ut in uts:
        ps = p["psMM"].tile([128, 512], F32, tag="psMM", name="fps")
        for kp in range(KD // 2):
            nc.tensor.matmul(ps[:, :T], ufT[:, 2 * kp:2 * kp + 2, ut * 128:(ut + 1) * 128],
                             g.tfT[:, 2 * kp:2 * kp + 2, :], start=(kp == 0), stop=False,
                             perf_mode=DR)
        nc.tensor.matmul(ps[:, :T], g.four16[:, :], g.growhl[0][:], start=False, stop=False)
        nc.tensor.matmul(ps[:, :T], g.four16[:, :], g.growhl[1][:], start=False, stop=True)
        e32 = p["tmp32"].tile([128, 512], F32, tag="tmp32")
        nc.scalar.activation(e32[:, :T], ps[:, :T], AF.Exp,
                             scale=2.0 * TEMP / SC2, accum_out=g.acc8[:, ut:ut + 1])
        nc.vector.tensor_scalar(g.fin[:, ut, :], ps[:, :T], 2.0 * TEMP / SC2, 0.0,
                                op0=AOP.mult, op1=AOP.add)


def _emit_final(g):
    """One batched Ln over the 8 exp-accumulators, then subtract + store."""
    nc, p = g.nc, g.pools
    lse8 = p["pp"].tile([128, 8], F32, tag="lse8")
    nc.scalar.activation(lse8[:, :], g.acc8[:, :], AF.Ln)
    for ut in range(U // 128):
        ot = p["tmp32"].tile([128, 512], F32, tag="tmp32")
        nc.vector.tensor_scalar(ot[:, :T], g.fin[:, ut, :], lse8[:, ut:ut + 1], 0.0,
                                op0=AOP.subtract, op1=AOP.add)
        nc.sync.dma_start(g.out[ut * 128:(ut + 1) * 128, :], ot[:, :T])


def _enc_gen(g, pfx, S, V, out_into=None):
    """Generator emitting one transformer encoder for seq len S, yielding at
    phase boundaries so two encoders can be interleaved. Returns (via
    StopIteration.value) the SBUF tile holding the final feature-major
    activations xT [128, KD, S] (fp8, 16x)."""
    nc, p = g.nc, g.pools
    NT = S // 128
    dram = g.dram

    # --- token indices + attention key bias ---
    idx = p["cst"].tile([128, NT], I32, tag=f"idx_{pfx}")
    nc.sync.dma_start(idx[:], dram[f"{pfx}_tok"].rearrange("(t p) -> p t", p=128))
    kb = p["cst"].tile([128, NT], F32, tag=f"kb_{pfx}")
    nc.sync.dma_start(kb[:], dram[f"{pfx}_kb"].rearrange("(t p) -> p t", p=128))

    # --- embedding gather (token-major) + PE transpose to feature-major ---
    stag = f"s8_{pfx}"
    ytag = f"y16_{pfx}"
    xT = p["stream"].tile([128, KD, S], F8, tag=stag, name="xT")
    for st in range(NT):
        x0 = p["x0"].tile([128, D], F16, tag="x0")
        nc.gpsimd.indirect_dma_start(
            out=x0[:], out_offset=None, in_=dram[f"{pfx}_emb"][:],
            in_offset=bass.IndirectOffsetOnAxis(ap=idx[:, st:st + 1], axis=0))
        tp = p["psO"].tile([128, KD * 128], F16, tag="psO")
        for kd in range(KD):
            nc.tensor.transpose(tp[:, kd * 128:(kd + 1) * 128],
                                x0[:, kd * 128:(kd + 1) * 128], g.ident[:])
        nc.vector.tensor_copy(xT[:, :, st * 128:(st + 1) * 128],
                              tp[:].rearrange("p (k c) -> p k c", k=KD))
        yield

    def ln_apply(y, s_t, b_t, xn):
        """Generator. y [128, KD, S] fp16 (16x) -> layer-norm into the
        caller-allocated fp8 tile xn (16x, per-token stats). When s_t is None
        the affine scale/bias was folded into downstream weights host-side."""
        DS = D
        for n0, NB in _nblocks(S):
            st2 = p["psRow"].tile([33, NB], F32, tag="psRow")
            ss, sq = st2[0:1, :], st2[32:33, :]
            for kd in range(KD):
                nc.tensor.matmul(ss[:], g.ones_c16[:], y[:, kd, n0:n0 + NB],
                                 start=(kd == 0), stop=(kd == KD - 1))
            for kd in range(KD):
                t = p["tmp16"].tile([128, 512], F16, tag="tmp16")
                nc.vector.tensor_tensor(t[:, :NB], y[:, kd, n0:n0 + NB],
                                        y[:, kd, n0:n0 + NB], op=AOP.mult)
                nc.tensor.matmul(sq[:], g.ones_c16[:], t[:, :NB],
                                 start=(kd == 0), stop=(kd == KD - 1))
            # ss = 16*DS*mean, sq = 256*sum_DS(y^2); compute true-scale sd then
            # 16x-normalizing A/C rows (fp16 for 1-cycle broadcast matmuls)
            m2 = p["row"].tile([1, 512], F32, tag="row")
            nc.scalar.activation(m2[:, :NB], ss[:], AF.Square, scale=1.0 / DS)
            var = p["row"].tile([1, 512], F32, tag="row")
            nc.vector.scalar_tensor_tensor(var[:, :NB], in0=sq[:], scalar=1.0 / DS,
                                           in1=m2[:, :NB], op0=AOP.mult,
                                           op1=AOP.subtract)
            sd = p["row"].tile([1, 512], F32, tag="row")
            nc.scalar.activation(sd[:, :NB], var[:, :NB], AF.Sqrt,
                                 scale=1.0 / SC2, bias=g.eps[:, :1])
            a32 = p["row"].tile([1, 512], F32, tag="row")
            nc.vector.reciprocal_approx_fast(a32[:, :NB], sd[:, :NB])
            a_t = p["row16"].tile([1, 512], F16, tag="row16a", bufs=4)
            nc.vector.tensor_copy(a_t[:, :NB], a32[:, :NB])
            c_t = p["row16"].tile([1, 512], F16, tag="row16c", bufs=4)
            nc.vector.scalar_tensor_tensor(c_t[:, :NB], in0=ss[:], scalar=-1.0 / DS,
                                           in1=a32[:, :NB], op0=AOP.mult,
                                           op1=AOP.mult)
            A = p["psMM"].tile([128, 512], F32, tag="psMM")
            nc.tensor.matmul(A[:, :NB], g.ones_r16[:], a_t[:, :NB], start=True, stop=True)
            C = p["psMM"].tile([128, 512], F32, tag="psMM")
            nc.tensor.matmul(C[:, :NB], g.ones_r16[:], c_t[:, :NB], start=True, stop=True)
            A16 = p["tmp16"].tile([128, 512], F16, tag="tagA16", bufs=2)
            nc.vector.tensor_copy(A16[:, :NB], A[:, :NB])
            for kd in range(KD):
                t1 = p["tmp16"].tile([128, 512], F16, tag="tmp16")
                nc.vector.tensor_tensor(t1[:, :NB], y[:, kd, n0:n0 + NB],
                                        A16[:, :NB], op=AOP.mult)
                if s_t is None:
                    nc.vector.tensor_tensor(xn[:, kd, n0:n0 + NB], t1[:, :NB],
                                            C[:, :NB], op=AOP.add)
                else:
                    t2 = p["tmp16"].tile([128, 512], F16, tag="tmp16")
                    nc.vector.tensor_tensor(t2[:, :NB], t1[:, :NB], C[:, :NB], op=AOP.add)
                    nc.scalar.activation(xn[:, kd, n0:n0 + NB], t2[:, :NB], AF.Identity,
                                         scale=s_t[:, kd:kd + 1], bias=b_t[:, kd:kd + 1])
            yield

    for l in range(L):
        w_r = dram[f"{pfx}w_{l}"]          # [16, 128, KD, 128] tiled (q,k cols)
        wv_r = dram[f"{pfx}wv_{l}"]        # [128, KD, D] contiguous
        ow_r = dram[f"{pfx}ow_{l}"]        # [8, 128, H, 128]
        f1_r = dram[f"{pfx}f1_{l}"]        # [16, 128, KD, 128]
        f2_r = dram[f"{pfx}f2_{l}"]        # [8, 128, KF, 128]
        ipb_t = p["pp"].tile([128, 24], F32, tag="ipb")
        nc.sync.dma_start(ipb_t[:], dram[f"{pfx}ipb_{l}"].rearrange("(j p) -> p j", p=128))
        vb_t = p["pp"].tile([128, H], F32, tag="vb")
        nc.sync.dma_start(vb_t[:], dram[f"{pfx}vb_{l}"].rearrange("(h p) -> p h", p=128))
        f1b_t = p["pp"].tile([128, KF], F32, tag="f1b")
        nc.sync.dma_start(f1b_t[:], dram[f"{pfx}f1b_{l}"].rearrange("(j p) -> p j", p=128))
        ls1_t = p["pp"].tile([128, KD], F32, tag="ls1")
        nc.sync.dma_start(ls1_t[:], dram[f"{pfx}ls1_{l}"].rearrange("(j p) -> p j", p=128))
        ps_t = p["pp"].tile([128, KD], F32, tag="ps")
        nc.sync.dma_start(ps_t[:], dram[f"{pfx}ps_{l}"].rearrange("(j p) -> p j", p=128))
        rb1 = p["row16"].tile([1, D], F16, tag="rb1", bufs=2)
        nc.sync.dma_start(rb1[:], dram[f"{pfx}rb1_{l}"][None, :])
        rb2 = p["row16"].tile([1, D], F16, tag="rb2", bufs=2)
        nc.sync.dma_start(rb2[:], dram[f"{pfx}rb2_{l}"][None, :])
        if l == L - 1:
            ls2_t = p["pp"].tile([128, KD], F32, tag="ls2")
            nc.sync.dma_start(ls2_t[:], dram[f"{pfx}ls2_{l}"].rearrange("(j p) -> p j", p=128))
            lb2_t = p["pp"].tile([128, KD], F32, tag="lb2")
            nc.sync.dma_start(lb2_t[:], dram[f"{pfx}lb2_{l}"].rearrange("(j p) -> p j", p=128))

        # ---- V projection (token-major, ones-augmented: v4[tok, st, h, 0:128]
        #      = 16*v (pre-bias), v4[..,128]=1 so the o-matmul accumulates
        #      softmax sums for free; v bias is folded into the oT copy) ----
        v4 = p["v"].tile([128, NT, H, DH + 1], F8, tag=f"v_{pfx}")
        nc.vector.memset(v4[:, :, :, DH:DH + 1], 1.0)
        wv = p["wv"].tile([128, KD, D], F8, tag=f"wv_{pfx}")
        nc.sync.dma_start(wv[:], wv_r[:])
        for st in range(NT):
            ps0 = p["psMM"].tile([128, 512], F32, tag="psMM")
            ps1 = p["psMM"].tile([128, 512], F32, tag="psMM")
            for kp in range(KD // 2):
                lh = xT[:, 2 * kp:2 * kp + 2, st * 128:(st + 1) * 128]
                nc.tensor.matmul(ps0[:], lh, wv[:, 2 * kp:2 * kp + 2, 0:512],
                                 start=(kp == 0), stop=(kp == KD // 2 - 1), perf_mode=DR)
                nc.tensor.matmul(ps1[:], lh, wv[:, 2 * kp:2 * kp + 2, 512:1024],
                                 start=(kp == 0), stop=(kp == KD // 2 - 1), perf_mode=DR)
            nc.vector.tensor_scalar(v4[:, st, 0:4, 0:DH],
                                    ps0[:].rearrange("p (a b) -> p a b", a=4),
                                    1.0 / SC, 0.0, op0=AOP.mult, op1=AOP.add)
            nc.vector.tensor_scalar(v4[:, st, 4:8, 0:DH],
                                    ps1[:].rearrange("p (a b) -> p a b", a=4),
                                    1.0 / SC, 0.0, op0=AOP.mult, op1=AOP.add)
        yield

        # ---- attention heads ----
        oT = p["oT"].tile([128, H, S], F8, tag=f"oT_{pfx}")
        for h in range(H):
            qk = p["qk"].tile([128, 2, S], F16, tag=f"qk_{pfx}")
            for part in range(2):
                wq = p["wcol"].tile([128, KD, 128], F8, tag=f"wcol_{pfx}")
                nc.sync.dma_start(wq[:], w_r[part * 8 + h])
                for pair in _npairs(S):
                    pss = [p["psMM"].tile([128, 512], F32, tag="psMM", name="psp") for _ in pair]
                    for kp in range(KD // 2):
                        for ps, (n0, NB) in zip(pss, pair):
                            nc.tensor.matmul(ps[:, :NB], wq[:, 2 * kp:2 * kp + 2, :],
                                             xT[:, 2 * kp:2 * kp + 2, n0:n0 + NB],
                                             start=(kp == 0), stop=(kp == KD // 2 - 1),
                                             perf_mode=DR)
                    for ps, (n0, NB) in zip(pss, pair):
                        nc.vector.tensor_scalar(qk[:, part, n0:n0 + NB], ps[:, :NB],
                                                1.0 / SC,
                                                ipb_t[:, part * 8 + h:part * 8 + h + 1],
                                                op0=AOP.mult, op1=AOP.add)
            attn = p["big"].tile([128, NT, S], F8, tag=f"big_{pfx}")
            for kt in range(NT):
                for pair in _npairs(S):
                    pss = [p["psMM"].tile([128, 512], F32, tag="psMM", name="psp") for _ in pair]
                    for ps, (n0, NB) in zip(pss, pair):
                        nc.tensor.matmul(ps[:, :NB], qk[:, 1, kt * 128:(kt + 1) * 128],
                                         qk[:, 0, n0:n0 + NB], start=True, stop=True)
                    for ps, (n0, NB) in zip(pss, pair):
                        nc.scalar.activation(attn[:, kt, n0:n0 + NB], ps[:, :NB], AF.Exp,
                                             scale=SCALE / SC2, bias=kb[:, kt:kt + 1])
            for qt in range(NT):
                po = p["psO"].tile([128, DH + 1], F32, tag="psO")
                for kt in range(NT):
                    nc.tensor.matmul(po[:], attn[:, kt, qt * 128:(qt + 1) * 128],
                                     v4[:, kt, h, :], start=(kt == 0), stop=(kt == NT - 1))
                rcol = p["pp"].tile([128, 1], F32, tag="rcol")
                nc.vector.reciprocal_approx_fast(rcol[:, :1], po[:, DH:DH + 1])
                osb = p["tmp16"].tile([128, 512], F16, tag="osb")
                nc.vector.tensor_scalar(osb[:, :DH], po[:, 0:DH], rcol[:, :1], 1.0 / SC,
                                        op0=AOP.mult, op1=AOP.mult)
                tps = p["psO"].tile([128, DH], F16, tag="psO")
                nc.tensor.transpose(tps[:], osb[:, :DH], g.ident[:])
                nc.vector.tensor_scalar(oT[:, h, qt * 128:(qt + 1) * 128], tps[:],
                                        vb_t[:, h:h + 1], 0.0, op0=AOP.add, op1=AOP.add)
            yield

        # ---- O projection + residual (y = x*prev_ln_s + (oproj + rb1), 16x) ----
        y = p["resid"].tile([128, KD, S], F16, tag=ytag, name="y")
        for m in range(KD):
            wo = p["wcol"].tile([128, H, 128], F8, tag=f"wcol_{pfx}")
            nc.sync.dma_start(wo[:], ow_r[m])
            for pair in _npairs(S):
                pss = [p["psMM"].tile([128, 512], F32, tag="psMM", name="psp") for _ in pair]
                for kp in range(H // 2):
                    for ps, (n0, NB) in zip(pss, pair):
                        nc.tensor.matmul(ps[:, :NB], wo[:, 2 * kp:2 * kp + 2, :],
                                         oT[:, 2 * kp:2 * kp + 2, n0:n0 + NB],
                                         start=(kp == 0), stop=False, perf_mode=DR)
                for ps, (n0, NB) in zip(pss, pair):
                    nc.tensor.matmul(ps[:, :NB], rb1[:, m * 128:(m + 1) * 128],
                                     g.ones_rw16[:, :NB], start=False, stop=True)
                    nc.vector.scalar_tensor_tensor(y[:, m, n0:n0 + NB],
                                                   in0=xT[:, m, n0:n0 + NB],
                                                   scalar=ps_t[:, m:m + 1],
                                                   in1=ps[:, :NB],
                                                   op0=AOP.mult, op1=AOP.add)
        yield

        xln = p["stream"].tile([128, KD, S], F8, tag=stag, name="xln")
        yield from ln_apply(y, None, None, xln)

        # ---- feed-forward + residual (y2 = xln*ln1_s + (ff2 + rb2), 16x) ----
        y2 = p["resid"].tile([128, KD, S], F16, tag=ytag, name="y2")
        for pair in _npairs(S):
            ffbs = [p["big"].tile([128, KF, min(512, S)], F8, tag=f"ffb_{pfx}", name="ffb", bufs=2 if pfx == "u" else 1) for _ in pair]
            for m in range(KF):
                wf = p["wcol"].tile([128, KD, 128], F8, tag=f"wcol_{pfx}")
                nc.sync.dma_start(wf[:], f1_r[m])
                pss = [p["psMM"].tile([128, 512], F32, tag="psMM", name="psp") for _ in pair]
                for kp in range(KD // 2):
                    for ps, (n0, NB) in zip(pss, pair):
                        nc.tensor.matmul(ps[:, :NB], wf[:, 2 * kp:2 * kp + 2, :],
                                         xln[:, 2 * kp:2 * kp + 2, n0:n0 + NB],
                                         start=(kp == 0), stop=(kp == KD // 2 - 1),
                                         perf_mode=DR)
                for ps, ffb, (n0, NB) in zip(pss, ffbs, pair):
                    nc.scalar.activation(ffb[:, m, :NB], ps[:, :NB], AF.Relu,
                                         scale=1.0 / SC2, bias=f1b_t[:, m:m + 1])
            yield
            for m2 in range(KD):
                wf2 = p["wcol2"].tile([128, KF, 128], F8, tag=f"wcol2_{pfx}")
                nc.sync.dma_start(wf2[:], f2_r[m2])
                pss = [p["psMM"].tile([128, 512], F32, tag="psMM", name="psp") for _ in pair]
                for kp in range(KF // 2):
                    for ps, ffb, (n0, NB) in zip(pss, ffbs, pair):
                        nc.tensor.matmul(ps[:, :NB], wf2[:, 2 * kp:2 * kp + 2, :],
                                         ffb[:, 2 * kp:2 * kp + 2, :NB],
                                         start=(kp == 0), stop=False, perf_mode=DR)
                for ps, (n0, NB) in zip(pss, pair):
                    nc.tensor.matmul(ps[:, :NB], rb2[:, m2 * 128:(m2 + 1) * 128],
                                     g.ones_rw16[:, :NB], start=False, stop=True)
                    nc.vector.scalar_tensor_tensor(y2[:, m2, n0:n0 + NB],
                                                   in0=xln[:, m2, n0:n0 + NB],
                                                   scalar=ls1_t[:, m2:m2 + 1],
                                                   in1=ps[:, :NB],
                                                   op0=AOP.mult, op1=AOP.add)
            yield

        if l == L - 1:
            dst = out_into if out_into is not None else (p["stream"], stag)
            xT = dst[0].tile([128, KD, S], F8, tag=dst[1], name="xnf")
            if pfx == "u":
                # drive the final LN block-by-block, emitting the cross/
                # log-softmax for u-tiles as soon as their LN block is done
                # (fills the final LN's DVE-bound tail with PE work)
                ln_gen = ln_apply(y2, ls2_t, lb2_t, xT)
                blocks = 0
                while True:
                    try:
                        next(ln_gen)
                    except StopIteration:
                        break
                    blocks += 1
                    if blocks == 1:
                        _emit_cross(g, xT, range(0, 4))
                    yield
                _emit_cross(g, xT, range(4, 8))
                _emit_final(g)
            else:
                yield from ln_apply(y2, ls2_t, lb2_t, xT)
        else:
            xT = p["stream"].tile([128, KD, S], F8, tag=stag, name="xn2")
            yield from ln_apply(y2, None, None, xT)
    return xT


def _build(Vt, Vu):
    nc = bacc.Bacc("TRN2", target_bir_lowering=False, debug=False, num_devices=B)
    dram = {}
    dram["t_tok"] = nc.dram_tensor("t_tok", [T], I32, kind="ExternalInput").ap()
    dram["u_tok"] = nc.dram_tensor("u_tok", [U], I32, kind="ExternalInput").ap()
    dram["t_emb"] = nc.dram_tensor("t_emb", [Vt, D], F16, kind="ExternalInput").ap()
    dram["u_emb"] = nc.dram_tensor("u_emb", [Vu, D], F16, kind="ExternalInput").ap()
    dram["t_kb"] = nc.dram_tensor("t_kb", [T], F32, kind="ExternalInput").ap()
    dram["u_kb"] = nc.dram_tensor("u_kb", [U], F32, kind="ExternalInput").ap()
    dram["tmask"] = nc.dram_tensor("tmask", [T], F32, kind="ExternalInput").ap()
    for e in ("t", "u"):
        for l in range(L):
            for name, shape, dt in [
                (f"{e}w_{l}", [16, 128, KD, 128], F8),
                (f"{e}wv_{l}", [128, KD, D], F8),
                (f"{e}ipb_{l}", [3 * D], F32),
                (f"{e}vb_{l}", [D], F32),
                (f"{e}ow_{l}", [8, 128, H, 128], F8),
                (f"{e}f1_{l}", [16, 128, KD, 128], F8), (f"{e}f1b_{l}", [FFD], F32),
                (f"{e}f2_{l}", [8, 128, KF, 128], F8),
                (f"{e}ls1_{l}", [D], F32), (f"{e}ps_{l}", [D], F32),
                (f"{e}rb1_{l}", [D], F16), (f"{e}rb2_{l}", [D], F16),
                (f"{e}ls2_{l}", [D], F32), (f"{e}lb2_{l}", [D], F32),
            ]:
                dram[name] = nc.dram_tensor(name, shape, dt, kind="ExternalInput").ap()
    out = nc.dram_tensor("out", [U, T], F32, kind="ExternalOutput").ap()

    g = _Ctx()
    g.nc = nc
    g.dram = dram

    with tile.TileContext(nc) as tc:
        with ExitStack() as es:
            p = {}
            p["cst"] = es.enter_context(tc.tile_pool(name="cst", bufs=1))
            p["stream"] = es.enter_context(tc.tile_pool(name="stream", bufs=3))
            p["resid"] = es.enter_context(tc.tile_pool(name="resid", bufs=1))
            p["tf"] = es.enter_context(tc.tile_pool(name="tf", bufs=1))
            p["qk"] = es.enter_context(tc.tile_pool(name="qk", bufs=2))
            p["v"] = es.enter_context(tc.tile_pool(name="v", bufs=1))
            p["big"] = es.enter_context(tc.tile_pool(name="big", bufs=2))
            p["oT"] = es.enter_context(tc.tile_pool(name="oT", bufs=1))
            p["tmp32"] = es.enter_context(tc.tile_pool(name="tmp32", bufs=4))
            p["tmp16"] = es.enter_context(tc.tile_pool(name="tmp16", bufs=3))
            p["x0"] = es.enter_context(tc.tile_pool(name="x0", bufs=3))
            p["wcol"] = es.enter_context(tc.tile_pool(name="wcol", bufs=4))
            p["wcol2"] = es.enter_context(tc.tile_pool(name="wcol2", bufs=2))
            p["wv"] = es.enter_context(tc.tile_pool(name="wv", bufs=1))
            p["row"] = es.enter_context(tc.tile_pool(name="row", bufs=4))
            p["row16"] = es.enter_context(tc.tile_pool(name="row16", bufs=2))
            p["pp"] = es.enter_context(tc.tile_pool(name="pp", bufs=2))
            p["psMM"] = es.enter_context(tc.tile_pool(name="psMM", bufs=4, space="PSUM"))
            p["psO"] = es.enter_context(tc.tile_pool(name="psO", bufs=2, space="PSUM"))
            p["psRow"] = es.enter_context(tc.tile_pool(name="psRow", bufs=2, space="PSUM"))
            g.pools = p

            g.ident = p["cst"].tile([128, 128], F16, tag="ident")
            make_identity(nc, g.ident[:])
            g.ones_c16 = p["cst"].tile([128, 1], F16, tag="oc16")
            nc.vector.memset(g.ones_c16[:], 1.0)
            g.ones_r16 = p["cst"].tile([1, 128], F16, tag="or16")
            nc.vector.memset(g.ones_r16[:], 1.0)
            g.ones_rw16 = p["cst"].tile([1, 512], F16, tag="orw16")
            nc.vector.memset(g.ones_rw16[:], 1.0)
            g.ones_r32 = p["cst"].tile([1, 128], F32, tag="or32")
            nc.vector.memset(g.ones_r32[:], 1.0)
            g.eps = p["cst"].tile([1, 1], F32, tag="eps")
            nc.vector.memset(g.eps[:], LN_EPS)
            g.four16 = p["cst"].tile([1, 128], F16, tag="four16")
            nc.vector.memset(g.four16[:], 4.0)
            g.acc8 = p["cst"].tile([128, 8], F32, tag="acc8")
            g.fin = p["cst"].tile([128, 8, 256], F16, tag="fin")

            # interleave the two encoders at phase granularity (unit first —
            # text work fills the unit encoder's pipeline stalls)
            g.out = out
            g.tfT = g.growhl = None
            gen_u = _enc_gen(g, "u", U, Vu)
            gen_t = _enc_gen(g, "t", T, Vt, out_into=(p["tf"], "tf"))
            done_u = done_t = False
            # stagger: prime the unit encoder ~half a layer ahead so the two
            # encoders' LayerNorm stalls don't synchronize in the PE queue
            for _ in range(7):
                next(gen_u)
            while not (done_u and done_t):
                if not done_u:
                    try:
                        next(gen_u)
                    except StopIteration:
                        done_u = True
                if not done_t:
                    try:
                        next(gen_t)
                    except StopIteration as e:
                        tfT, done_t = e.value, True
                        # tn[t] = sum_d tf^2 (256x); g_row = -0.5*tn + tmask
                        tmask_t = p["row"].tile([1, 512], F32, tag="row")
                        nc.sync.dma_start(tmask_t[:, :T], dram["tmask"][None, :])
                        tnp = p["psRow"].tile([33, T], F32, tag="psRow")
                        for kd in range(KD):
                            t = p["tmp16"].tile([128, 512], F16, tag="tmp16")
                            nc.vector.tensor_tensor(t[:, :T], tfT[:, kd, :],
                                                    tfT[:, kd, :], op=AOP.mult)
                            nc.tensor.matmul(tnp[0:1, :], g.ones_c16[:], t[:, :T],
                                             start=(kd == 0), stop=(kd == KD - 1))
                        grow = p["cst"].tile([1, T], F32, tag="grow")
                        nc.vector.scalar_tensor_tensor(grow[:], in0=tnp[0:1, :], scalar=-0.5,
                                                       in1=tmask_t[:, :T],
                                                       op0=AOP.mult, op1=AOP.add)
                        # hi/lo fp16 split of the grow row: the bias add into
                        # the 256x cross psum becomes one cheap K=2 fp16
                        # matmul instead of a 4-cycle/row fp32 one
                        # rows stored at 1/4 scale (grow ~ 1.5e5 overflows
                        # fp16); the stationary ones-row carries the 4x back
                        ghi = p["cst"].tile([1, T], F16, tag="ghi")
                        nc.vector.tensor_scalar(ghi[:], grow[:], 0.25, 0.0,
                                                op0=AOP.mult, op1=AOP.add)
                        glo = p["cst"].tile([1, T], F16, tag="glo")
                        nc.vector.scalar_tensor_tensor(glo[:], in0=grow[:], scalar=0.25,
                                                       in1=ghi[:], op0=AOP.mult,
                                                       op1=AOP.subtract)
                        g.tfT, g.growhl = tfT, (ghi, glo)

    nc.compile()
    return nc


def _tile_w(wT):
    """[Din, C] -> [C//128, 128, Din//128, 128]: each output [cb] block is the
    contiguous SBUF image of one stationary-column load (p, k, c)."""
    Din, C = wT.shape
    t = wT.reshape(Din // 128, 128, C // 128, 128)   # (k, p, cb, c)
    return np.ascontiguousarray(t.transpose(2, 1, 0, 3))  # (cb, p, k, c)


def _f8(a):
    return np.clip(a, -240.0, 240.0).astype(NP_F8)


def _prep_in_maps(inputs):
    f32 = np.float32
    tok_t = np.ascontiguousarray(np.asarray(inputs["text_tokens"]).astype(np.int32))
    tok_u = np.ascontiguousarray(np.asarray(inputs["unit_tokens"]).astype(np.int32))
    Vt = inputs["t_emb"].shape[0]
    Vu = inputs["u_emb"].shape[0]

    shared = {
        "t_emb": np.ascontiguousarray(
            (np.asarray(inputs["t_emb"], f32) * SC).astype(np.float16)),
        "u_emb": np.ascontiguousarray(
            (np.asarray(inputs["u_emb"], f32) * SC).astype(np.float16)),
    }
    for e in ("t", "u"):
        pf = e + "_"
        # LayerNorm affine folding: each non-final LN's scale/bias is folded
        # into the next consumer's weights/biases (exact when scale=1, bias=0).
        prev_s = np.ones(D, f32)
        prev_b = np.zeros(D, f32)
        for l in range(L):
            ipw = np.asarray(inputs[pf + "ipw"][l], f32)
            ipb = np.asarray(inputs[pf + "ipb"][l], f32)
            ow = np.asarray(inputs[pf + "ow"][l], f32)
            ob = np.asarray(inputs[pf + "ob"][l], f32)
            f1w = np.asarray(inputs[pf + "f1w"][l], f32)
            f1b = np.asarray(inputs[pf + "f1b"][l], f32)
            f2w = np.asarray(inputs[pf + "f2w"][l], f32)
            f2b = np.asarray(inputs[pf + "f2b"][l], f32)
            l1s = np.asarray(inputs[pf + "l1s"][l], f32)
            l1b = np.asarray(inputs[pf + "l1b"][l], f32)
            wT = (ipw * prev_s[None, :]).T * SC      # [Din, 3D], 16x
            shared[f"{e}w_{l}"] = _f8(_tile_w(wT[:, :2 * D]))
            shared[f"{e}wv_{l}"] = _f8(np.ascontiguousarray(
                wT[:, 2 * D:].reshape(KD, 128, D).transpose(1, 0, 2)))
            bfull = ipb + ipw @ prev_b
            # q/k bias 16x (qk tile is 16x); v bias true-scale (added to oT)
            shared[f"{e}ipb_{l}"] = np.ascontiguousarray(
                np.concatenate([bfull[:2 * D] * SC, bfull[2 * D:]]))
            shared[f"{e}vb_{l}"] = np.ascontiguousarray(bfull[2 * D:])
            shared[f"{e}ow_{l}"] = _f8(_tile_w(ow.T * SC))
            shared[f"{e}ps_{l}"] = np.ascontiguousarray(prev_s)
            shared[f"{e}rb1_{l}"] = np.ascontiguousarray(
                ((ob + prev_b) * SC).astype(np.float16))
            shared[f"{e}f1_{l}"] = _f8(_tile_w((f1w * l1s[None, :]).T * SC))
            shared[f"{e}f1b_{l}"] = np.ascontiguousarray(f1b + f1w @ l1b)
            shared[f"{e}f2_{l}"] = _f8(_tile_w(f2w.T * SC))
            shared[f"{e}rb2_{l}"] = np.ascontiguousarray(
                ((f2b + l1b) * SC).astype(np.float16))
            shared[f"{e}ls1_{l}"] = np.ascontiguousarray(l1s)
            shared[f"{e}ls2_{l}"] = np.ascontiguousarray(np.asarray(inputs[pf + "l2s"][l], f32))
            shared[f"{e}lb2_{l}"] = np.ascontiguousarray(
                np.asarray(inputs[pf + "l2b"][l], f32) * SC)
            prev_s = np.asarray(inputs[pf + "l2s"][l], f32)
            prev_b = np.asarray(inputs[pf + "l2b"][l], f32)

    in_maps = []
    for c in range(B):
        tpad = tok_t[c] == Vt - 1
        upad = tok_u[c] == Vu - 1
        m = dict(shared)
        m["t_tok"] = tok_t[c]
        m["u_tok"] = tok_u[c]
        m["t_kb"] = np.where(tpad, np.float32(NEG), np.float32(0.0)).astype(f32)
        m["u_kb"] = np.where(upad, np.float32(NEG), np.float32(0.0)).astype(f32)
        m["tmask"] = np.where(tpad, np.float32(NEG * SC2 / (2.0 * TEMP)),
                              np.float32(0.0)).astype(f32)
        in_maps.append(m)
    return in_maps, tok_u, Vu


def kernel(**inputs):
    global LAST_RESULTS
    Vt = inputs["t_emb"].shape[0]
    Vu = inputs["u_emb"].shape[0]
    key = (Vt, Vu)
    if key not in _CACHE:
        _CACHE[key] = _build(Vt, Vu)
    nc = _CACHE[key]

    in_maps, tok_u, Vu_ = _prep_in_maps(inputs)
    kw = {}
    if TRACE:
        kw = dict(trace=True, tmpdir=TRACE_DIR)
    br = run_bass_kernel_spmd(nc, in_maps, list(range(B)), **kw)
    LAST_RESULTS = br

    out = np.stack([br.results[c]["out"] for c in range(B)], axis=0)
    # padded unit rows: reference gives uniform -log(T) rows (never triggers
    # with the spec's token distribution, but exact when it does)
    for c in range(B):
        upad = tok_u[c] == Vu_ - 1
        if upad.any():
            out[c, upad, :] = -np.log(np.float32(T)).astype(np.float32)
    return out
